# revision 34
# baseline (speedup 1.0000x reference)
"""Trainium2 Bass kernel for causal multi-head attention with partial RoPE.

Problem (nn_Attn): x[128,512,384] -> qkv proj (Wqkv [1152,384]) -> split
q,k,v into 6 heads of 64 -> partial rotary (first 16 channels) on q,k ->
causal softmax attention -> out proj (Wout [384,384]).

Strategy: data-parallel over batch B across 8 NeuronCores (16 batches per
core). Per batch, on each core:
  A. load x [512,384], PE-transpose to xT [384,512] (f32r)
  B. qkT = Wqk @ xT (d-major layout: 6 tiles of [128,512], 2 heads per
     tile), plus a "swap" projection qB = Wswap @ xT which realizes the
     RoPE channel permutation (with signs) inside the matmul. RoPE is
     then qkT = qA*cos_mask + qB*sin_mask (elementwise, DVE).
     v = xT.T @ WvT in [t, d] layout.
  C. attention per head-pair p (2 heads stacked in one 128-tile):
     S^T chunks [k=128, q<=512] via row-packed K=64 matmuls; causal
     diagonal masked by preloading -1e30 into PSUM via identity@maskW
     matmul before accumulation; exp on ACT (bf16 out);
     O^T[hd, q] accumulated via col-packed K=128 matmuls with V;
     softmax denominators via ones-column matmuls; normalization
     r = exp(-ln(sum)) broadcast across partitions with K=1 matmuls.
  D. y = O^T.T @ WoutT in [t, c] layout, DMA out.

Projections use float32r (TF32-like, 1 cycle/row for N>=256); q/k tiles,
P and V use bf16 (1 cycle/row at any N); accumulation is fp32 in PSUM.
Emission is software-pipelined: batch b+1's projection phase is emitted
before batch b's attention so the tensor engine never idles long enough
to drop out of the HAM K=8/8 (2.4 GHz) clock state.
"""

import os
import sys

for _p in ("/opt/trn_rl_repo",):
    if _p not in sys.path and os.path.isdir(_p):
        sys.path.append(_p)

import numpy as np
import ml_dtypes

import concourse.bass as bass
import concourse.mybir as mybir
import concourse.tile as tile
from concourse.bass import ts
from concourse.bass_utils import run_bass_kernel_spmd
from concourse.vector_clock import ScopedClock

B, T, C, NH, RD = 128, 512, 384, 6, 16
HD = C // NH  # 64
NCORES = 8
BL = B // NCORES  # batches per core
NDT = 2 * C // 128  # 6 qk d-tiles
NTT = T // 128  # 4 t-tiles
NPAIR = NH // 2  # 3 head pairs
F32 = mybir.dt.float32
F32R = mybir.dt.float32r
BF16 = mybir.dt.bfloat16
AF = mybir.ActivationFunctionType
NEG = -1e30


import bass_rust as _bass_rust


class TC(tile.TileContext):
    """TileContext adapted to this walrus build, which rejects more than
    one sync-wait command on an instruction: excess waits are split onto
    same-engine NoOps inserted immediately before the instruction."""

    MAX_WAITS = 1

    def _lower_ordered_insts(self, ordered):
        for bb_name, insts in list(ordered.items()):
            out = []
            for inst in insts:
                si = getattr(inst, "sync_info", None)
                waits = list(si.on_wait) if si is not None and si.on_wait else []
                if len(waits) > self.MAX_WAITS:
                    extra = waits[: -self.MAX_WAITS]
                    si.on_wait = waits[-self.MAX_WAITS:]
                    for w in extra:
                        n = _bass_rust.InstNoOp(
                            name=self.nc.get_next_instruction_name(),
                            ins=[], outs=[], engine=inst.engine,
                        )
                        n.sync_info = mybir.SyncInfo(on_wait=[w], on_update=[])
                        out.append(n)
                out.append(inst)
            ordered[bb_name] = out
        super()._lower_ordered_insts(ordered)

    def _drain_and_barrier(self, tick_clock, wait_clock):
        drain_inst = self.nc.sync.drain()
        wait_clock.add_sem_waits(
            drain_inst.ins, ScopedClock({None: tick_clock.global_clock})
        )
        waits = list(drain_inst.ins.sync_info.on_wait)
        if len(waits) > 1:
            drain_inst.ins.sync_info.on_wait = [waits[0]]
            for w in waits[1:]:
                n = self.nc.sync.nop(nofuse=True)
                n.ins.sync_info = mybir.SyncInfo(on_wait=[w], on_update=[])
            self.nc.sync.drain()
        self.nc.all_engine_barrier()
        popped = self.nc._tile_sem_poison_stack.pop()
        assert popped is self._sem_poison
        self.nc.clear_and_free_semaphores(list(self.sems.allocated().values()))
        self.nc.all_engine_barrier()


def build_program(bl=BL):
    nc = bass.Bass("TRN2", target_bir_lowering=False, num_devices=NCORES)

    x_d = nc.dram_tensor("x", [bl, T, C], F32, kind="ExternalInput")
    wqk_d = nc.dram_tensor("wqkT", [C, 2 * C], F32, kind="ExternalInput")
    wv_d = nc.dram_tensor("wvT", [C, C], F32, kind="ExternalInput")
    wo_d = nc.dram_tensor("woutT", [C, C], F32, kind="ExternalInput")
    cm_d = nc.dram_tensor("cmask", [128, T], F32, kind="ExternalInput")
    sm_d = nc.dram_tensor("smask", [128, T], F32, kind="ExternalInput")
    mw_d = nc.dram_tensor("maskw", [128, 128], BF16, kind="ExternalInput")
    idb_d = nc.dram_tensor("identb", [128, 128], BF16, kind="ExternalInput")
    idf_d = nc.dram_tensor("identf", [128, 128], F32, kind="ExternalInput")
    oca_d = nc.dram_tensor("onescol2a", [128, 2], BF16, kind="ExternalInput")
    ocb_d = nc.dram_tensor("onescol2b", [128, 2], BF16, kind="ExternalInput")
    bc2_d = nc.dram_tensor("bc2", [128, 128], F32, kind="ExternalInput")
    y_d = nc.dram_tensor("y", [bl, T, C], F32, kind="ExternalOutput")

    with TC(nc) as tc:
        _body(nc, tc, bl, x_d, wqk_d, wv_d, wo_d, cm_d, sm_d, mw_d,
              idb_d, idf_d, oca_d, ocb_d, bc2_d, y_d)
    return nc


def _body(nc, tc, bl, x_d, wqk_d, wv_d, wo_d, cm_d, sm_d, mw_d,
          idb_d, idf_d, oca_d, ocb_d, bc2_d, y_d):
    import contextlib

    ctx = contextlib.ExitStack()
    singles = ctx.enter_context(tc.tile_pool(name="singles", bufs=1))
    stage = ctx.enter_context(tc.tile_pool(name="stage", bufs=2))
    sb_x = ctx.enter_context(tc.tile_pool(name="sb_x", bufs=12))
    sb_xT = ctx.enter_context(tc.tile_pool(name="sb_xT", bufs=9))
    sb_qk = ctx.enter_context(tc.tile_pool(name="sb_qk", bufs=18))
    sb_rp = ctx.enter_context(tc.tile_pool(name="sb_rp", bufs=4))
    sb_v = ctx.enter_context(tc.tile_pool(name="sb_v", bufs=12))
    sb_p = ctx.enter_context(tc.tile_pool(name="sb_p", bufs=6))
    sb_r = ctx.enter_context(tc.tile_pool(name="sb_r", bufs=3))
    sb_ot = ctx.enter_context(tc.tile_pool(name="sb_ot", bufs=6))
    sb_y = ctx.enter_context(tc.tile_pool(name="sb_y", bufs=4))
    # PSUM: 8 banks of 2KB. ps_main [128,512]x2 = 2 banks (transposes, qk
    # projection, v projection, r-broadcast, out projection); ps_s
    # [128,1024]x2 = 4 banks (S^T pair tiles); ps_o 1 bank; ps_sum 1 bank.
    ps_main = ctx.enter_context(tc.tile_pool(name="ps_main", bufs=2, space="PSUM"))
    ps_s = ctx.enter_context(tc.tile_pool(name="ps_s", bufs=2, space="PSUM"))
    ps_o = ctx.enter_context(tc.tile_pool(name="ps_o", bufs=1, space="PSUM"))

    # ---- constants ----
    def load_const(dram, shape, dtype, tag):
        t = singles.tile(shape, dtype, tag=tag)
        nc.sync.dma_start(out=t, in_=dram.ap())
        return t

    cmask = load_const(cm_d, [128, T], F32, "cmask")
    smask_f = load_const(sm_d, [128, T], F32, "smask_f")
    smask = singles.tile([128, T], BF16, tag="smask")
    nc.vector.tensor_copy(out=smask, in_=smask_f)
    maskw = load_const(mw_d, [128, 128], BF16, "maskw")
    # maskw duplicated side by side: lets one matmul add the causal mask to
    # both heads' diagonal S^T blocks (out AP strided across the two banks)
    maskw2 = singles.tile([128, 2, 128], BF16, tag="maskw2")
    nc.vector.tensor_copy(out=maskw2[:, 0, :], in_=maskw)
    nc.vector.tensor_copy(out=maskw2[:, 1, :], in_=maskw)
    identb = load_const(idb_d, [128, 128], BF16, "identb")
    identf = load_const(idf_d, [128, 128], F32, "identf")
    onescol2a = load_const(oca_d, [128, 2], BF16, "onescol2a")
    bc2_st = load_const(bc2_d, [128, 128], F32, "bc2")
    bc2 = singles.tile([128, 128], F32R, tag="bc2_r")
    nc.vector.tensor_copy(out=bc2, in_=bc2_st)
    # persistent ln-row tile; rows 2-127 must be finite (zero) because the
    # K=128 broadcast matmul streams them against zero weights
    lnr = singles.tile([128, T], F32R, tag="lnr")
    nc.vector.memset(lnr.bitcast(F32), 0.0)

    # weights: stage fp32 then cast to bf16 (bf16 moving operands stream
    # at 2.4 GHz = 1 col/cycle; fp32/f32r moving streams at half rate)
    def load_w_bf16(dram, cols, name):
        outs = []
        for cc in range(3):
            st = stage.tile([128, cols], F32, tag="wstage")
            nc.sync.dma_start(out=st, in_=dram[ts(cc, 128), :])
            wr = singles.tile([128, cols], BF16, tag=f"{name}{cc}")
            nc.vector.tensor_copy(out=wr, in_=st)
            outs.append(wr)
        return outs

    wqk = load_w_bf16(wqk_d, 2 * C, "wqk")
    wv = load_w_bf16(wv_d, C, "wv")
    wo = load_w_bf16(wo_d, C, "wo")

    # RoPE swap permutation: rows p <-> p+8 for p%32 < 8 (within each
    # 32-block); applies the rotary channel pairing to rows 0-15/64-79,
    # with rotation signs folded into smask.
    shuf_mask = list(range(32))
    for i in range(8):
        shuf_mask[i], shuf_mask[i + 8] = shuf_mask[i + 8], shuf_mask[i]

    def emit_front(b):
        """x load, transpose, qk projection + rope, v projection."""
        xts = []
        for tt in range(NTT):
            xt = sb_x.tile([128, C], F32, tag="x")
            nc.sync.dma_start(out=xt, in_=x_d[b, ts(tt, 128), :])
            xts.append(xt)
        xT = []
        for cc in range(3):
            tp = ps_main.tile([128, 512], F32, tag="m")
            for tt in range(NTT):
                # 4 transposes into one PSUM bank: only the first may
                # start=True (start clears the whole 2KB zero-region)
                nc.tensor.matmul(tp[:, ts(tt, 128)],
                                 xts[tt][:, ts(cc, 128)],
                                 identf, is_transpose=True,
                                 start=(tt == 0), stop=(tt == NTT - 1),
                                 skip_group_check=True)
            xTt = sb_xT.tile([128, T], BF16, tag="xT")
            nc.vector.tensor_copy(out=xTt, in_=tp)
            xT.append(xTt)

        qkT = []
        for dt in range(NDT):
            qa = ps_main.tile([128, 512], F32, tag="m")
            for cc in range(3):
                nc.tensor.matmul(qa, wqk[cc][:, ts(dt, 128)], xT[cc],
                                 start=(cc == 0), stop=(cc == 2))
            t_sb = sb_rp.tile([128, T], BF16, tag="ropet")
            shf = sb_rp.tile([128, T], F32, tag="ropes")
            m_sb = sb_rp.tile([128, T], BF16, tag="ropem")
            nc.vector.tensor_mul(t_sb, qa, cmask)
            nc.vector.stream_shuffle(shf, qa, shuf_mask)
            nc.vector.tensor_mul(m_sb, shf, smask)
            qk = sb_qk.tile([128, T], BF16, tag="qk")
            nc.vector.tensor_add(qk, t_sb, m_sb)
            qkT.append(qk)

        vts = []
        for tt in range(NTT):
            vp = ps_main.tile([128, 512], F32, tag="m")
            for cc in range(3):
                nc.tensor.matmul(vp[:, 0:C], xT[cc][:, ts(tt, 128)], wv[cc],
                                 start=(cc == 0), stop=(cc == 2))
            # V-augmented layout [1 | V_h | 1] per head: even heads use
            # cols 1:66 ([V|1], sum row last), odd heads cols 0:65 ([1|V],
            # sum row first) so one O matmul also produces softmax sums
            vt = sb_v.tile([128, NH, 66], BF16, tag="v")
            nc.vector.memset(vt[:, :, 0], 1.0)
            nc.vector.memset(vt[:, :, 65], 1.0)
            vtv = vt[:, :, 1:65]
            vpv = vp[:, 0:C].rearrange("p (n d) -> p n d", n=NH)
            if tt == 3:
                nc.vector.tensor_copy(out=vtv, in_=vpv)
            else:
                nc.scalar.activation(vtv, vpv, AF.Copy)
            vts.append(vt)
        return qkT, vts

    def emit_back(b, qkT, vts):
        """Attention per head pair + output projection."""
        oTs = []
        for p in range(NPAIR):
            qt = qkT[p]
            kt = qkT[3 + p]
            ha, hb = 2 * p, 2 * p + 1
            # o2: head a in bank 0 (partitions 0-64, sum row at 64), head b
            # in bank 1 (partitions 63-127, sum row at 63)
            o2 = ps_o.tile([128, 1024], F32, tag="o")
            # phase 1: all S^T matmuls + exp, so the PE never head-of-line
            # blocks on an exp result (O matmuls are emitted afterwards)
            pts = []
            for j in range(NTT):
                qs = 128 * j
                w = T - qs
                st = ps_s.tile([128, 1024], F32, tag="s")
                # S^T row-packed matmuls (K=64 each head), then accumulate
                # the causal -1e30 mask onto the diagonal 128 cols
                st_view = st.rearrange("p (h f) -> p h f", h=2)
                nc.tensor.matmul(st[:, 0:w], kt[0:64, ts(j, 128)],
                                 qt[0:64, qs:T], start=True, stop=False,
                                 skip_group_check=True)
                nc.tensor.matmul(st[:, 512:512 + w], kt[64:128, ts(j, 128)],
                                 qt[64:128, qs:T], start=True, stop=False,
                                 skip_group_check=True)
                nc.tensor.matmul(st_view[:, :, 0:128], identb, maskw2,
                                 start=False, stop=True,
                                 skip_group_check=True)
                # exp (both heads in one ACT op), bf16 out
                pt = sb_p.tile([128, 2, 512], BF16, tag="p")
                nc.scalar.activation(pt[:, :, 0:w], st_view[:, :, 0:w], AF.Exp)
                pts.append(pt)
            # phase 2: O^T accumulation; the V-aug ones column makes the
            # same matmul emit the softmax denominators (rows 64 / 63)
            for j in range(NTT):
                qs = 128 * j
                w = T - qs
                pt = pts[j]
                nc.tensor.matmul(o2[0:65, qs:T], vts[j][:, ha, 1:66],
                                 pt[:, 0, 0:w], start=(j == 0), stop=(j == 3),
                                 skip_group_check=True)
                nc.tensor.matmul(o2[64:128, 512 + qs:512 + T],
                                 vts[j][:, hb, 1:65],
                                 pt[:, 1, 0:w], start=(j == 0), stop=(j == 3),
                                 skip_group_check=True)
                nc.tensor.matmul(o2[96:98, qs:T], onescol2a,
                                 pt[:, 1, 0:w], start=(j == 0), stop=(j == 3),
                                 skip_group_check=True, tile_position=(0, 96))
            # softmax denominator: r = exp(-ln(sum)) broadcast
            nc.scalar.activation(lnr[64:65, :], o2[64:65, 0:T], AF.Ln)
            nc.scalar.activation(lnr[96:97, :], o2[96:97, 0:T], AF.Ln)
            rb = ps_main.tile([128, 512], F32, tag="m")
            nc.tensor.matmul(rb[:, 0:T], bc2, lnr, start=True, stop=True)
            r_sb = sb_r.tile([128, T], F32, tag="r")
            nc.scalar.activation(r_sb, rb[:, 0:T], AF.Exp, scale=-1.0)
            ot = sb_ot.tile([128, T], BF16, tag="ot")
            nc.vector.tensor_mul(ot[0:64, :], o2[0:64, 0:T], r_sb[0:64, :])
            nc.vector.tensor_mul(ot[64:128, :], o2[64:128, 512:512 + T],
                                 r_sb[64:128, :])
            oTs.append(ot)

        for tt in range(NTT):
            yp = ps_main.tile([128, 512], F32, tag="m")
            for p in range(NPAIR):
                nc.tensor.matmul(yp[:, 0:C], oTs[p][:, ts(tt, 128)], wo[p],
                                 start=(p == 0), stop=(p == NPAIR - 1))
            yt = sb_y.tile([128, C], F32, tag="y")
            if tt % 2 == 0:
                nc.vector.tensor_copy(out=yt, in_=yp[:, 0:C])
            else:
                nc.scalar.activation(yt, yp[:, 0:C], AF.Copy)
            nc.sync.dma_start(out=y_d[b, ts(tt, 128), :], in_=yt)

    # software pipeline, 2 batches of lookahead: attention of batch b is
    # emitted BEFORE the projection phase of batch b+2, so the scheduler
    # prefers attention and uses the (already-emitted) front work of
    # batches b+1/b+2 as filler during exp/rope waits
    states = [emit_front(0), emit_front(1) if bl > 1 else None]
    for b in range(bl):
        emit_back(b, *states[b % 2])
        if b + 2 < bl:
            states[b % 2] = emit_front(b + 2)

    ctx.close()


def make_host_consts(Wqkv, Wout, cos, sin):
    Wq = Wqkv[0:C].astype(np.float32) / np.sqrt(np.float32(HD))
    Wk = Wqkv[C:2 * C].astype(np.float32)
    Wv = Wqkv[2 * C:3 * C].astype(np.float32)

    wqkT = np.concatenate([Wq, Wk], axis=0).T.copy()  # [C, 2C]

    wvT = Wv.T.copy()
    woutT = Wout.astype(np.float32).T.copy()

    cosA = np.asarray(cos, np.float32).reshape(T, RD // 2)  # [T, 8]
    sinA = np.asarray(sin, np.float32).reshape(T, RD // 2)
    cmask = np.ones((128, T), np.float32)
    smask = np.zeros((128, T), np.float32)
    for base in (0, 64):
        for i in range(RD):
            cmask[base + i] = cosA[:, i % (RD // 2)]
            # row i (i<8) holds r1_out = r1*cos - r2*sin; the shuffle swaps
            # in r2, so the sin factor is negative there
            sgn = -1.0 if i < RD // 2 else 1.0
            smask[base + i] = sgn * sinA[:, i % (RD // 2)]

    kk, qq = np.meshgrid(np.arange(128), np.arange(128), indexing="ij")
    maskw = np.where(qq >= kk, 0.0, NEG).astype(ml_dtypes.bfloat16)
    identb = np.eye(128, dtype=ml_dtypes.bfloat16)
    identf = np.eye(128, dtype=np.float32)
    onescol2a = np.zeros((128, 2), ml_dtypes.bfloat16)
    onescol2a[:, 0] = 1.0
    onescol2b = np.zeros((128, 2), ml_dtypes.bfloat16)
    onescol2b[:, 1] = 1.0
    # broadcast matrix for r = exp(-ln(sum)): head-a sums live in lnr row
    # 64, head-b sums in row 63 (V-aug layout)
    bc2 = np.zeros((128, 128), np.float32)
    bc2[64, 0:64] = 1.0
    bc2[96, 64:128] = 1.0

    return dict(wqkT=wqkT, wvT=wvT, woutT=woutT,
                cmask=cmask, smask=smask, maskw=maskw, identb=identb,
                identf=identf, onescol2a=onescol2a, onescol2b=onescol2b,
                bc2=bc2)


_CACHE = {}


def prepare(x, Wqkv, Wout, cos, sin):
    if "nc" not in _CACHE:
        _CACHE["nc"] = build_program()
    nc = _CACHE["nc"]
    consts = make_host_consts(np.asarray(Wqkv), np.asarray(Wout), cos, sin)
    x = np.ascontiguousarray(np.asarray(x, np.float32))
    in_maps = []
    for c in range(NCORES):
        m = dict(consts)
        m["x"] = x[c * BL:(c + 1) * BL]
        in_maps.append(m)
    return nc, in_maps


def run(x, Wqkv, Wout, cos, sin, trace=False):
    nc, in_maps = prepare(x, Wqkv, Wout, cos, sin)
    res = run_bass_kernel_spmd(
        nc, in_maps, core_ids=list(range(NCORES)), trace=trace
    )
    y = np.concatenate([res.results[c]["y"] for c in range(NCORES)], axis=0)
    return y, res


def kernel(x, Wqkv, Wout, cos, sin):
    y, _ = run(x, Wqkv, Wout, cos, sin, trace=False)
    return y



# revision 35
# speedup vs baseline: 1.5353x; 1.5353x over previous
"""Trainium2 Bass kernel for causal multi-head attention with partial RoPE.

Problem (nn_Attn): x[128,512,384] -> qkv proj (Wqkv [1152,384]) -> split
q,k,v into 6 heads of 64 -> partial rotary (first 16 channels) on q,k ->
causal softmax attention -> out proj (Wout [384,384]).

Strategy: data-parallel over batch B across 8 NeuronCores (16 batches per
core). Per batch, on each core:
  A. load x [512,384], PE-transpose to xT [384,512] (f32r)
  B. qkT = Wqk @ xT (d-major layout: 6 tiles of [128,512], 2 heads per
     tile), plus a "swap" projection qB = Wswap @ xT which realizes the
     RoPE channel permutation (with signs) inside the matmul. RoPE is
     then qkT = qA*cos_mask + qB*sin_mask (elementwise, DVE).
     v = xT.T @ WvT in [t, d] layout.
  C. attention per head-pair p (2 heads stacked in one 128-tile):
     S^T chunks [k=128, q<=512] via row-packed K=64 matmuls; causal
     diagonal masked by preloading -1e30 into PSUM via identity@maskW
     matmul before accumulation; exp on ACT (bf16 out);
     O^T[hd, q] accumulated via col-packed K=128 matmuls with V;
     softmax denominators via ones-column matmuls; normalization
     r = exp(-ln(sum)) broadcast across partitions with K=1 matmuls.
  D. y = O^T.T @ WoutT in [t, c] layout, DMA out.

Projections use float32r (TF32-like, 1 cycle/row for N>=256); q/k tiles,
P and V use bf16 (1 cycle/row at any N); accumulation is fp32 in PSUM.
Emission is software-pipelined: batch b+1's projection phase is emitted
before batch b's attention so the tensor engine never idles long enough
to drop out of the HAM K=8/8 (2.4 GHz) clock state.
"""

import os
import sys

for _p in ("/opt/trn_rl_repo",):
    if _p not in sys.path and os.path.isdir(_p):
        sys.path.append(_p)

import numpy as np
import ml_dtypes

import concourse.bass as bass
import concourse.mybir as mybir
import concourse.tile as tile
from concourse.bass import ts
from concourse.bass_utils import run_bass_kernel_spmd
from concourse.vector_clock import ScopedClock

B, T, C, NH, RD = 128, 512, 384, 6, 16
HD = C // NH  # 64
NCORES = 8
BL = B // NCORES  # batches per core
NDT = 2 * C // 128  # 6 qk d-tiles
NTT = T // 128  # 4 t-tiles
NPAIR = NH // 2  # 3 head pairs
F32 = mybir.dt.float32
F32R = mybir.dt.float32r
BF16 = mybir.dt.bfloat16
AF = mybir.ActivationFunctionType
NEG = -1e30


import bass_rust as _bass_rust


class TC(tile.TileContext):
    """TileContext adapted to this walrus build, which rejects more than
    one sync-wait command on an instruction: excess waits are split onto
    same-engine NoOps inserted immediately before the instruction."""

    MAX_WAITS = 1

    def _lower_ordered_insts(self, ordered):
        for bb_name, insts in list(ordered.items()):
            out = []
            for inst in insts:
                si = getattr(inst, "sync_info", None)
                waits = list(si.on_wait) if si is not None and si.on_wait else []
                if len(waits) > self.MAX_WAITS:
                    extra = waits[: -self.MAX_WAITS]
                    si.on_wait = waits[-self.MAX_WAITS:]
                    for w in extra:
                        n = _bass_rust.InstNoOp(
                            name=self.nc.get_next_instruction_name(),
                            ins=[], outs=[], engine=inst.engine,
                        )
                        n.sync_info = mybir.SyncInfo(on_wait=[w], on_update=[])
                        out.append(n)
                out.append(inst)
            ordered[bb_name] = out
        super()._lower_ordered_insts(ordered)

    def _drain_and_barrier(self, tick_clock, wait_clock):
        drain_inst = self.nc.sync.drain()
        wait_clock.add_sem_waits(
            drain_inst.ins, ScopedClock({None: tick_clock.global_clock})
        )
        waits = list(drain_inst.ins.sync_info.on_wait)
        if len(waits) > 1:
            drain_inst.ins.sync_info.on_wait = [waits[0]]
            for w in waits[1:]:
                n = self.nc.sync.nop(nofuse=True)
                n.ins.sync_info = mybir.SyncInfo(on_wait=[w], on_update=[])
            self.nc.sync.drain()
        self.nc.all_engine_barrier()
        popped = self.nc._tile_sem_poison_stack.pop()
        assert popped is self._sem_poison
        self.nc.clear_and_free_semaphores(list(self.sems.allocated().values()))
        self.nc.all_engine_barrier()


def build_program(bl=BL):
    nc = bass.Bass("TRN2", target_bir_lowering=False, num_devices=NCORES)

    x_d = nc.dram_tensor("x", [bl, T, C], F32, kind="ExternalInput")
    wqk_d = nc.dram_tensor("wqkT", [C, 2 * C], F32, kind="ExternalInput")
    wv_d = nc.dram_tensor("wvT", [C, C], F32, kind="ExternalInput")
    wo_d = nc.dram_tensor("woutT", [C, C], F32, kind="ExternalInput")
    cm_d = nc.dram_tensor("cmask", [128, T], F32, kind="ExternalInput")
    sm_d = nc.dram_tensor("smask", [128, T], F32, kind="ExternalInput")
    mw_d = nc.dram_tensor("maskw", [128, 128], BF16, kind="ExternalInput")
    idb_d = nc.dram_tensor("identb", [128, 128], BF16, kind="ExternalInput")
    idf_d = nc.dram_tensor("identf", [128, 128], F32, kind="ExternalInput")
    oca_d = nc.dram_tensor("onescol2a", [128, 2], BF16, kind="ExternalInput")
    ocb_d = nc.dram_tensor("onescol2b", [128, 2], BF16, kind="ExternalInput")
    bc2_d = nc.dram_tensor("bc2", [128, 128], F32, kind="ExternalInput")
    y_d = nc.dram_tensor("y", [bl, T, C], F32, kind="ExternalOutput")

    with TC(nc) as tc:
        _body(nc, tc, bl, x_d, wqk_d, wv_d, wo_d, cm_d, sm_d, mw_d,
              idb_d, idf_d, oca_d, ocb_d, bc2_d, y_d)
    return nc


def _body(nc, tc, bl, x_d, wqk_d, wv_d, wo_d, cm_d, sm_d, mw_d,
          idb_d, idf_d, oca_d, ocb_d, bc2_d, y_d):
    import contextlib

    ctx = contextlib.ExitStack()
    singles = ctx.enter_context(tc.tile_pool(name="singles", bufs=1))
    stage = ctx.enter_context(tc.tile_pool(name="stage", bufs=2))
    sb_x = ctx.enter_context(tc.tile_pool(name="sb_x", bufs=12))
    sb_xT = ctx.enter_context(tc.tile_pool(name="sb_xT", bufs=9))
    sb_qk = ctx.enter_context(tc.tile_pool(name="sb_qk", bufs=18))
    sb_rp = ctx.enter_context(tc.tile_pool(name="sb_rp", bufs=4))
    sb_v = ctx.enter_context(tc.tile_pool(name="sb_v", bufs=12))
    sb_p = ctx.enter_context(tc.tile_pool(name="sb_p", bufs=6))
    sb_r = ctx.enter_context(tc.tile_pool(name="sb_r", bufs=3))
    sb_ot = ctx.enter_context(tc.tile_pool(name="sb_ot", bufs=6))
    sb_y = ctx.enter_context(tc.tile_pool(name="sb_y", bufs=4))
    # PSUM: 8 banks of 2KB. ps_main [128,512]x2 = 2 banks (transposes, qk
    # projection, v projection, r-broadcast, out projection); ps_s
    # [128,1024]x2 = 4 banks (S^T pair tiles); ps_o 1 bank; ps_sum 1 bank.
    ps_main = ctx.enter_context(tc.tile_pool(name="ps_main", bufs=2, space="PSUM"))
    ps_s = ctx.enter_context(tc.tile_pool(name="ps_s", bufs=2, space="PSUM"))
    ps_o = ctx.enter_context(tc.tile_pool(name="ps_o", bufs=1, space="PSUM"))
    ps_sum = ctx.enter_context(tc.tile_pool(name="ps_sum", bufs=1, space="PSUM"))

    # ---- constants ----
    def load_const(dram, shape, dtype, tag):
        t = singles.tile(shape, dtype, tag=tag)
        nc.sync.dma_start(out=t, in_=dram.ap())
        return t

    cmask = load_const(cm_d, [128, T], F32, "cmask")
    smask_f = load_const(sm_d, [128, T], F32, "smask_f")
    smask = singles.tile([128, T], BF16, tag="smask")
    nc.vector.tensor_copy(out=smask, in_=smask_f)
    maskw = load_const(mw_d, [128, 128], BF16, "maskw")
    # maskw duplicated side by side: lets one matmul add the causal mask to
    # both heads' diagonal S^T blocks (out AP strided across the two banks)
    maskw2 = singles.tile([128, 2, 128], BF16, tag="maskw2")
    nc.vector.tensor_copy(out=maskw2[:, 0, :], in_=maskw)
    nc.vector.tensor_copy(out=maskw2[:, 1, :], in_=maskw)
    identb = load_const(idb_d, [128, 128], BF16, "identb")
    identf = load_const(idf_d, [128, 128], F32, "identf")
    onescol2a = load_const(oca_d, [128, 2], BF16, "onescol2a")
    onescol2b = load_const(ocb_d, [128, 2], BF16, "onescol2b")
    bc2_st = load_const(bc2_d, [128, 128], F32, "bc2")
    bc2 = singles.tile([128, 128], F32R, tag="bc2_r")
    nc.vector.tensor_copy(out=bc2, in_=bc2_st)
    # persistent ln-row tile; rows 2-127 must be finite (zero) because the
    # K=128 broadcast matmul streams them against zero weights
    lnr = singles.tile([128, T], F32R, tag="lnr")
    nc.vector.memset(lnr.bitcast(F32), 0.0)

    # weights: stage fp32 then cast to bf16 (bf16 moving operands stream
    # at 2.4 GHz = 1 col/cycle; fp32/f32r moving streams at half rate)
    def load_w_bf16(dram, cols, name):
        outs = []
        for cc in range(3):
            st = stage.tile([128, cols], F32, tag="wstage")
            nc.sync.dma_start(out=st, in_=dram[ts(cc, 128), :])
            wr = singles.tile([128, cols], BF16, tag=f"{name}{cc}")
            nc.vector.tensor_copy(out=wr, in_=st)
            outs.append(wr)
        return outs

    wqk = load_w_bf16(wqk_d, 2 * C, "wqk")
    wv = load_w_bf16(wv_d, C, "wv")
    wo = load_w_bf16(wo_d, C, "wo")

    # RoPE swap permutation: rows p <-> p+8 for p%32 < 8 (within each
    # 32-block); applies the rotary channel pairing to rows 0-15/64-79,
    # with rotation signs folded into smask.
    shuf_mask = list(range(32))
    for i in range(8):
        shuf_mask[i], shuf_mask[i + 8] = shuf_mask[i + 8], shuf_mask[i]

    def emit_front(b):
        """x load, transpose, qk projection + rope, v projection."""
        xts = []
        for tt in range(NTT):
            xt = sb_x.tile([128, C], F32, tag="x")
            nc.sync.dma_start(out=xt, in_=x_d[b, ts(tt, 128), :])
            xts.append(xt)
        xT = []
        for cc in range(3):
            tp = ps_main.tile([128, 512], F32, tag="m")
            for tt in range(NTT):
                # 4 transposes into one PSUM bank: only the first may
                # start=True (start clears the whole 2KB zero-region)
                nc.tensor.matmul(tp[:, ts(tt, 128)],
                                 xts[tt][:, ts(cc, 128)],
                                 identf, is_transpose=True,
                                 start=(tt == 0), stop=(tt == NTT - 1),
                                 skip_group_check=True)
            xTt = sb_xT.tile([128, T], BF16, tag="xT")
            nc.vector.tensor_copy(out=xTt, in_=tp)
            xT.append(xTt)

        qkT = []
        for dt in range(NDT):
            qa = ps_main.tile([128, 512], F32, tag="m")
            for cc in range(3):
                nc.tensor.matmul(qa, wqk[cc][:, ts(dt, 128)], xT[cc],
                                 start=(cc == 0), stop=(cc == 2))
            t_sb = sb_rp.tile([128, T], BF16, tag="ropet")
            shf = sb_rp.tile([128, T], F32, tag="ropes")
            m_sb = sb_rp.tile([128, T], BF16, tag="ropem")
            nc.vector.tensor_mul(t_sb, qa, cmask)
            nc.vector.stream_shuffle(shf, qa, shuf_mask)
            nc.vector.tensor_mul(m_sb, shf, smask)
            qk = sb_qk.tile([128, T], BF16, tag="qk")
            nc.vector.tensor_add(qk, t_sb, m_sb)
            qkT.append(qk)

        vts = []
        for tt in range(NTT):
            vp = ps_main.tile([128, 512], F32, tag="m")
            for cc in range(3):
                nc.tensor.matmul(vp[:, 0:C], xT[cc][:, ts(tt, 128)], wv[cc],
                                 start=(cc == 0), stop=(cc == 2))
            vt = sb_v.tile([128, C], BF16, tag="v")
            if tt == 3:
                nc.vector.tensor_copy(out=vt, in_=vp[:, 0:C])
            else:
                nc.scalar.activation(vt, vp[:, 0:C], AF.Copy)
            vts.append(vt)
        return qkT, vts

    def emit_back(b, qkT, vts):
        """Attention per head pair + output projection."""
        oTs = []
        for p in range(NPAIR):
            qt = qkT[p]
            kt = qkT[3 + p]
            ha, hb = 2 * p, 2 * p + 1
            o_ps = ps_o.tile([128, T], F32, tag="o")
            s_ps = ps_sum.tile([128, T], F32, tag="sum")
            # phase 1: all S^T matmuls + exp, so the PE never head-of-line
            # blocks on an exp result (O matmuls are emitted afterwards)
            pts = []
            for j in range(NTT):
                qs = 128 * j
                w = T - qs
                st = ps_s.tile([128, 1024], F32, tag="s")
                # S^T row-packed matmuls (K=64 each head), then accumulate
                # the causal -1e30 mask onto the diagonal 128 cols
                st_view = st.rearrange("p (h f) -> p h f", h=2)
                nc.tensor.matmul(st[:, 0:w], kt[0:64, ts(j, 128)],
                                 qt[0:64, qs:T], start=True, stop=False,
                                 skip_group_check=True)
                nc.tensor.matmul(st[:, 512:512 + w], kt[64:128, ts(j, 128)],
                                 qt[64:128, qs:T], start=True, stop=False,
                                 skip_group_check=True)
                nc.tensor.matmul(st_view[:, :, 0:128], identb, maskw2,
                                 start=False, stop=True,
                                 skip_group_check=True)
                # exp (both heads in one ACT op), bf16 out
                pt = sb_p.tile([128, 2, 512], BF16, tag="p")
                nc.scalar.activation(pt[:, :, 0:w], st_view[:, :, 0:w], AF.Exp)
                pts.append(pt)
            # phase 2: O^T accumulation (col-packed) + sums
            for j in range(NTT):
                qs = 128 * j
                w = T - qs
                pt = pts[j]
                nc.tensor.matmul(o_ps[0:64, qs:T], vts[j][:, ts(ha, HD)],
                                 pt[:, 0, 0:w], start=(j == 0), stop=(j == 3),
                                 skip_group_check=True)
                nc.tensor.matmul(o_ps[64:128, qs:T], vts[j][:, ts(hb, HD)],
                                 pt[:, 1, 0:w], start=(j == 0), stop=(j == 3),
                                 skip_group_check=True)
                nc.tensor.matmul(s_ps[0:2, qs:T], onescol2a,
                                 pt[:, 0, 0:w], start=(j == 0), stop=False,
                                 skip_group_check=True)
                nc.tensor.matmul(s_ps[0:2, qs:T], onescol2b,
                                 pt[:, 1, 0:w], start=False, stop=(j == 3),
                                 skip_group_check=True)
            # softmax denominator: r = exp(-ln(sum)) broadcast
            nc.scalar.activation(lnr[0:2, :], s_ps[0:2, :], AF.Ln)
            rb = ps_main.tile([128, 512], F32, tag="m")
            nc.tensor.matmul(rb[:, 0:T], bc2, lnr, start=True, stop=True)
            r_sb = sb_r.tile([128, T], F32, tag="r")
            nc.scalar.activation(r_sb, rb[:, 0:T], AF.Exp, scale=-1.0)
            ot = sb_ot.tile([128, T], BF16, tag="ot")
            nc.vector.tensor_mul(ot, o_ps, r_sb)
            oTs.append(ot)

        for tt in range(NTT):
            yp = ps_main.tile([128, 512], F32, tag="m")
            for p in range(NPAIR):
                nc.tensor.matmul(yp[:, 0:C], oTs[p][:, ts(tt, 128)], wo[p],
                                 start=(p == 0), stop=(p == NPAIR - 1))
            yt = sb_y.tile([128, C], F32, tag="y")
            if tt % 2 == 0:
                nc.vector.tensor_copy(out=yt, in_=yp[:, 0:C])
            else:
                nc.scalar.activation(yt, yp[:, 0:C], AF.Copy)
            nc.sync.dma_start(out=y_d[b, ts(tt, 128), :], in_=yt)

    # software pipeline, 2 batches of lookahead: attention of batch b is
    # emitted BEFORE the projection phase of batch b+2, so the scheduler
    # prefers attention and uses the (already-emitted) front work of
    # batches b+1/b+2 as filler during exp/rope waits
    states = [emit_front(0), emit_front(1) if bl > 1 else None]
    for b in range(bl):
        emit_back(b, *states[b % 2])
        if b + 2 < bl:
            states[b % 2] = emit_front(b + 2)

    ctx.close()


def make_host_consts(Wqkv, Wout, cos, sin):
    Wq = Wqkv[0:C].astype(np.float32) / np.sqrt(np.float32(HD))
    Wk = Wqkv[C:2 * C].astype(np.float32)
    Wv = Wqkv[2 * C:3 * C].astype(np.float32)

    wqkT = np.concatenate([Wq, Wk], axis=0).T.copy()  # [C, 2C]

    wvT = Wv.T.copy()
    woutT = Wout.astype(np.float32).T.copy()

    cosA = np.asarray(cos, np.float32).reshape(T, RD // 2)  # [T, 8]
    sinA = np.asarray(sin, np.float32).reshape(T, RD // 2)
    cmask = np.ones((128, T), np.float32)
    smask = np.zeros((128, T), np.float32)
    for base in (0, 64):
        for i in range(RD):
            cmask[base + i] = cosA[:, i % (RD // 2)]
            # row i (i<8) holds r1_out = r1*cos - r2*sin; the shuffle swaps
            # in r2, so the sin factor is negative there
            sgn = -1.0 if i < RD // 2 else 1.0
            smask[base + i] = sgn * sinA[:, i % (RD // 2)]

    kk, qq = np.meshgrid(np.arange(128), np.arange(128), indexing="ij")
    maskw = np.where(qq >= kk, 0.0, NEG).astype(ml_dtypes.bfloat16)
    identb = np.eye(128, dtype=ml_dtypes.bfloat16)
    identf = np.eye(128, dtype=np.float32)
    onescol2a = np.zeros((128, 2), ml_dtypes.bfloat16)
    onescol2a[:, 0] = 1.0
    onescol2b = np.zeros((128, 2), ml_dtypes.bfloat16)
    onescol2b[:, 1] = 1.0
    bc2 = np.zeros((128, 128), np.float32)
    bc2[0, 0:64] = 1.0
    bc2[1, 64:128] = 1.0

    return dict(wqkT=wqkT, wvT=wvT, woutT=woutT,
                cmask=cmask, smask=smask, maskw=maskw, identb=identb,
                identf=identf, onescol2a=onescol2a, onescol2b=onescol2b,
                bc2=bc2)


_CACHE = {}


def prepare(x, Wqkv, Wout, cos, sin):
    if "nc" not in _CACHE:
        _CACHE["nc"] = build_program()
    nc = _CACHE["nc"]
    consts = make_host_consts(np.asarray(Wqkv), np.asarray(Wout), cos, sin)
    x = np.ascontiguousarray(np.asarray(x, np.float32))
    in_maps = []
    for c in range(NCORES):
        m = dict(consts)
        m["x"] = x[c * BL:(c + 1) * BL]
        in_maps.append(m)
    return nc, in_maps


def run(x, Wqkv, Wout, cos, sin, trace=False):
    nc, in_maps = prepare(x, Wqkv, Wout, cos, sin)
    res = run_bass_kernel_spmd(
        nc, in_maps, core_ids=list(range(NCORES)), trace=trace
    )
    y = np.concatenate([res.results[c]["y"] for c in range(NCORES)], axis=0)
    return y, res


def kernel(x, Wqkv, Wout, cos, sin):
    y, _ = run(x, Wqkv, Wout, cos, sin, trace=False)
    return y



# revision 36
# speedup vs baseline: 1.5745x; 1.0255x over previous
"""Trainium2 Bass kernel for causal multi-head attention with partial RoPE.

Problem (nn_Attn): x[128,512,384] -> qkv proj (Wqkv [1152,384]) -> split
q,k,v into 6 heads of 64 -> partial rotary (first 16 channels) on q,k ->
causal softmax attention -> out proj (Wout [384,384]).

Strategy: data-parallel over batch B across 8 NeuronCores (16 batches per
core). Per batch, on each core:
  A. load x [512,384], PE-transpose to xT [384,512] (f32r)
  B. qkT = Wqk @ xT (d-major layout: 6 tiles of [128,512], 2 heads per
     tile), plus a "swap" projection qB = Wswap @ xT which realizes the
     RoPE channel permutation (with signs) inside the matmul. RoPE is
     then qkT = qA*cos_mask + qB*sin_mask (elementwise, DVE).
     v = xT.T @ WvT in [t, d] layout.
  C. attention per head-pair p (2 heads stacked in one 128-tile):
     S^T chunks [k=128, q<=512] via row-packed K=64 matmuls; causal
     diagonal masked by preloading -1e30 into PSUM via identity@maskW
     matmul before accumulation; exp on ACT (bf16 out);
     O^T[hd, q] accumulated via col-packed K=128 matmuls with V;
     softmax denominators via ones-column matmuls; normalization
     r = exp(-ln(sum)) broadcast across partitions with K=1 matmuls.
  D. y = O^T.T @ WoutT in [t, c] layout, DMA out.

Projections use float32r (TF32-like, 1 cycle/row for N>=256); q/k tiles,
P and V use bf16 (1 cycle/row at any N); accumulation is fp32 in PSUM.
Emission is software-pipelined: batch b+1's projection phase is emitted
before batch b's attention so the tensor engine never idles long enough
to drop out of the HAM K=8/8 (2.4 GHz) clock state.
"""

import os
import sys

for _p in ("/opt/trn_rl_repo",):
    if _p not in sys.path and os.path.isdir(_p):
        sys.path.append(_p)

import numpy as np
import ml_dtypes

import concourse.bass as bass
import concourse.mybir as mybir
import concourse.tile as tile
from concourse.bass import ts
from concourse.bass_utils import run_bass_kernel_spmd
from concourse.vector_clock import ScopedClock

B, T, C, NH, RD = 128, 512, 384, 6, 16
HD = C // NH  # 64
NCORES = 8
BL = B // NCORES  # batches per core
NDT = 2 * C // 128  # 6 qk d-tiles
NTT = T // 128  # 4 t-tiles
NPAIR = NH // 2  # 3 head pairs
F32 = mybir.dt.float32
F32R = mybir.dt.float32r
BF16 = mybir.dt.bfloat16
AF = mybir.ActivationFunctionType
NEG = -1e30


import bass_rust as _bass_rust


class TC(tile.TileContext):
    """TileContext adapted to this walrus build, which rejects more than
    one sync-wait command on an instruction: excess waits are split onto
    same-engine NoOps inserted immediately before the instruction."""

    MAX_WAITS = 1

    def _lower_ordered_insts(self, ordered):
        for bb_name, insts in list(ordered.items()):
            out = []
            for inst in insts:
                si = getattr(inst, "sync_info", None)
                waits = list(si.on_wait) if si is not None and si.on_wait else []
                if len(waits) > self.MAX_WAITS:
                    extra = waits[: -self.MAX_WAITS]
                    si.on_wait = waits[-self.MAX_WAITS:]
                    for w in extra:
                        n = _bass_rust.InstNoOp(
                            name=self.nc.get_next_instruction_name(),
                            ins=[], outs=[], engine=inst.engine,
                        )
                        n.sync_info = mybir.SyncInfo(on_wait=[w], on_update=[])
                        out.append(n)
                out.append(inst)
            ordered[bb_name] = out
        super()._lower_ordered_insts(ordered)

    def _drain_and_barrier(self, tick_clock, wait_clock):
        drain_inst = self.nc.sync.drain()
        wait_clock.add_sem_waits(
            drain_inst.ins, ScopedClock({None: tick_clock.global_clock})
        )
        waits = list(drain_inst.ins.sync_info.on_wait)
        if len(waits) > 1:
            drain_inst.ins.sync_info.on_wait = [waits[0]]
            for w in waits[1:]:
                n = self.nc.sync.nop(nofuse=True)
                n.ins.sync_info = mybir.SyncInfo(on_wait=[w], on_update=[])
            self.nc.sync.drain()
        self.nc.all_engine_barrier()
        popped = self.nc._tile_sem_poison_stack.pop()
        assert popped is self._sem_poison
        self.nc.clear_and_free_semaphores(list(self.sems.allocated().values()))
        self.nc.all_engine_barrier()


def build_program(bl=BL):
    nc = bass.Bass("TRN2", target_bir_lowering=False, num_devices=NCORES)

    x_d = nc.dram_tensor("x", [bl, T, C], F32, kind="ExternalInput")
    wqk_d = nc.dram_tensor("wqkT", [C, 2 * C], F32, kind="ExternalInput")
    wv_d = nc.dram_tensor("wvT", [C, C], F32, kind="ExternalInput")
    wo_d = nc.dram_tensor("woutT", [C, C], F32, kind="ExternalInput")
    cm_d = nc.dram_tensor("cmask", [128, T], F32, kind="ExternalInput")
    sm_d = nc.dram_tensor("smask", [128, T], F32, kind="ExternalInput")
    mw_d = nc.dram_tensor("maskw", [128, 128], BF16, kind="ExternalInput")
    idb_d = nc.dram_tensor("identb", [128, 128], BF16, kind="ExternalInput")
    idf_d = nc.dram_tensor("identf", [128, 128], F32, kind="ExternalInput")
    oca_d = nc.dram_tensor("onescol2a", [128, 2], BF16, kind="ExternalInput")
    ocb_d = nc.dram_tensor("onescol2b", [128, 2], BF16, kind="ExternalInput")
    bc2_d = nc.dram_tensor("bc2", [128, 128], F32, kind="ExternalInput")
    y_d = nc.dram_tensor("y", [bl, T, C], F32, kind="ExternalOutput")

    with TC(nc) as tc:
        _body(nc, tc, bl, x_d, wqk_d, wv_d, wo_d, cm_d, sm_d, mw_d,
              idb_d, idf_d, oca_d, ocb_d, bc2_d, y_d)
    return nc


def _body(nc, tc, bl, x_d, wqk_d, wv_d, wo_d, cm_d, sm_d, mw_d,
          idb_d, idf_d, oca_d, ocb_d, bc2_d, y_d):
    import contextlib

    ctx = contextlib.ExitStack()
    singles = ctx.enter_context(tc.tile_pool(name="singles", bufs=1))
    stage = ctx.enter_context(tc.tile_pool(name="stage", bufs=2))
    sb_x = ctx.enter_context(tc.tile_pool(name="sb_x", bufs=12))
    sb_xT = ctx.enter_context(tc.tile_pool(name="sb_xT", bufs=9))
    sb_qk = ctx.enter_context(tc.tile_pool(name="sb_qk", bufs=18))
    sb_rp = ctx.enter_context(tc.tile_pool(name="sb_rp", bufs=4))
    sb_v = ctx.enter_context(tc.tile_pool(name="sb_v", bufs=12))
    sb_p = ctx.enter_context(tc.tile_pool(name="sb_p", bufs=6))
    sb_r = ctx.enter_context(tc.tile_pool(name="sb_r", bufs=3))
    sb_ot = ctx.enter_context(tc.tile_pool(name="sb_ot", bufs=6))
    sb_y = ctx.enter_context(tc.tile_pool(name="sb_y", bufs=4))
    # PSUM: 8 banks of 2KB. ps_main [128,512]x2 = 2 banks (transposes, qk
    # projection, v projection, r-broadcast, out projection); ps_s
    # [128,1024]x2 = 4 banks (S^T pair tiles); ps_o 1 bank; ps_sum 1 bank.
    ps_main = ctx.enter_context(tc.tile_pool(name="ps_main", bufs=2, space="PSUM"))
    ps_s = ctx.enter_context(tc.tile_pool(name="ps_s", bufs=2, space="PSUM"))
    ps_o = ctx.enter_context(tc.tile_pool(name="ps_o", bufs=1, space="PSUM"))
    ps_sum = ctx.enter_context(tc.tile_pool(name="ps_sum", bufs=1, space="PSUM"))

    # ---- constants ----
    def load_const(dram, shape, dtype, tag):
        t = singles.tile(shape, dtype, tag=tag)
        nc.sync.dma_start(out=t, in_=dram.ap())
        return t

    cmask = load_const(cm_d, [128, T], F32, "cmask")
    smask_f = load_const(sm_d, [128, T], F32, "smask_f")
    smask = singles.tile([128, T], BF16, tag="smask")
    nc.vector.tensor_copy(out=smask, in_=smask_f)
    maskw = load_const(mw_d, [128, 128], BF16, "maskw")
    # maskw duplicated side by side: lets one matmul add the causal mask to
    # both heads' diagonal S^T blocks (out AP strided across the two banks)
    maskw2 = singles.tile([128, 2, 128], BF16, tag="maskw2")
    nc.vector.tensor_copy(out=maskw2[:, 0, :], in_=maskw)
    nc.vector.tensor_copy(out=maskw2[:, 1, :], in_=maskw)
    identb = load_const(idb_d, [128, 128], BF16, "identb")
    identf = load_const(idf_d, [128, 128], F32, "identf")
    onescol2a = load_const(oca_d, [128, 2], BF16, "onescol2a")
    onescol2b = load_const(ocb_d, [128, 2], BF16, "onescol2b")
    bc2_st = load_const(bc2_d, [128, 128], F32, "bc2")
    bc2 = singles.tile([128, 128], F32R, tag="bc2_r")
    nc.vector.tensor_copy(out=bc2, in_=bc2_st)
    # persistent ln-row tile; rows 2-127 must be finite (zero) because the
    # K=128 broadcast matmul streams them against zero weights
    lnr = singles.tile([128, T], F32R, tag="lnr")
    nc.vector.memset(lnr.bitcast(F32), 0.0)

    # weights: stage fp32 then cast to bf16 (bf16 moving operands stream
    # at 2.4 GHz = 1 col/cycle; fp32/f32r moving streams at half rate)
    def load_w_bf16(dram, cols, name):
        outs = []
        for cc in range(3):
            st = stage.tile([128, cols], F32, tag="wstage")
            nc.sync.dma_start(out=st, in_=dram[ts(cc, 128), :])
            wr = singles.tile([128, cols], BF16, tag=f"{name}{cc}")
            nc.vector.tensor_copy(out=wr, in_=st)
            outs.append(wr)
        return outs

    wqk = load_w_bf16(wqk_d, 2 * C, "wqk")
    wv = load_w_bf16(wv_d, C, "wv")
    wo = load_w_bf16(wo_d, C, "wo")

    # RoPE swap permutation: rows p <-> p+8 for p%32 < 8 (within each
    # 32-block); applies the rotary channel pairing to rows 0-15/64-79,
    # with rotation signs folded into smask.
    shuf_mask = list(range(32))
    for i in range(8):
        shuf_mask[i], shuf_mask[i + 8] = shuf_mask[i + 8], shuf_mask[i]

    def emit_front(b):
        """x load, transpose, qk projection + rope, v projection."""
        xts = []
        for tt in range(NTT):
            xt = sb_x.tile([128, C], F32, tag="x")
            nc.sync.dma_start(out=xt, in_=x_d[b, ts(tt, 128), :])
            xts.append(xt)
        xT = []
        for cc in range(3):
            tp = ps_main.tile([128, 512], F32, tag="m")
            for tt in range(NTT):
                # 4 transposes into one PSUM bank: only the first may
                # start=True (start clears the whole 2KB zero-region)
                nc.tensor.matmul(tp[:, ts(tt, 128)],
                                 xts[tt][:, ts(cc, 128)],
                                 identf, is_transpose=True,
                                 start=(tt == 0), stop=(tt == NTT - 1),
                                 skip_group_check=True)
            xTt = sb_xT.tile([128, T], BF16, tag="xT")
            nc.vector.tensor_copy(out=xTt, in_=tp)
            xT.append(xTt)

        qkT = []
        for dt in range(NDT):
            qa = ps_main.tile([128, 512], F32, tag="m")
            for cc in range(3):
                nc.tensor.matmul(qa, wqk[cc][:, ts(dt, 128)], xT[cc],
                                 start=(cc == 0), stop=(cc == 2))
            t_sb = sb_rp.tile([128, T], BF16, tag="ropet")
            shf = sb_rp.tile([128, T], F32, tag="ropes")
            m_sb = sb_rp.tile([128, T], BF16, tag="ropem")
            nc.vector.tensor_mul(t_sb, qa, cmask)
            nc.vector.stream_shuffle(shf, qa, shuf_mask)
            nc.vector.tensor_mul(m_sb, shf, smask)
            qk = sb_qk.tile([128, T], BF16, tag="qk")
            nc.vector.tensor_add(qk, t_sb, m_sb)
            qkT.append(qk)

        vts = []
        for tt in range(NTT):
            vp = ps_main.tile([128, 512], F32, tag="m")
            for cc in range(3):
                nc.tensor.matmul(vp[:, 0:C], xT[cc][:, ts(tt, 128)], wv[cc],
                                 start=(cc == 0), stop=(cc == 2))
            vt = sb_v.tile([128, C], BF16, tag="v")
            nc.scalar.activation(vt, vp[:, 0:C], AF.Copy)
            vts.append(vt)
        return qkT, vts

    def emit_back(b, qkT, vts):
        """Attention per head pair + output projection."""
        oTs = []
        for p in range(NPAIR):
            qt = qkT[p]
            kt = qkT[3 + p]
            ha, hb = 2 * p, 2 * p + 1
            o_ps = ps_o.tile([128, T], F32, tag="o")
            s_ps = ps_sum.tile([128, T], F32, tag="sum")
            # phase 1: all S^T matmuls + exp, so the PE never head-of-line
            # blocks on an exp result (O matmuls are emitted afterwards)
            pts = []
            for j in range(NTT):
                qs = 128 * j
                w = T - qs
                st = ps_s.tile([128, 1024], F32, tag="s")
                # S^T row-packed matmuls (K=64 each head), then accumulate
                # the causal -1e30 mask onto the diagonal 128 cols
                st_view = st.rearrange("p (h f) -> p h f", h=2)
                nc.tensor.matmul(st[:, 0:w], kt[0:64, ts(j, 128)],
                                 qt[0:64, qs:T], start=True, stop=False,
                                 skip_group_check=True)
                nc.tensor.matmul(st[:, 512:512 + w], kt[64:128, ts(j, 128)],
                                 qt[64:128, qs:T], start=True, stop=False,
                                 skip_group_check=True)
                nc.tensor.matmul(st_view[:, :, 0:128], identb, maskw2,
                                 start=False, stop=True,
                                 skip_group_check=True)
                # exp (both heads in one ACT op), bf16 out
                pt = sb_p.tile([128, 2, 512], BF16, tag="p")
                nc.scalar.activation(pt[:, :, 0:w], st_view[:, :, 0:w], AF.Exp)
                pts.append(pt)
            # phase 2: O^T accumulation (col-packed) + sums
            for j in range(NTT):
                qs = 128 * j
                w = T - qs
                pt = pts[j]
                nc.tensor.matmul(o_ps[0:64, qs:T], vts[j][:, ts(ha, HD)],
                                 pt[:, 0, 0:w], start=(j == 0), stop=(j == 3),
                                 skip_group_check=True)
                nc.tensor.matmul(o_ps[64:128, qs:T], vts[j][:, ts(hb, HD)],
                                 pt[:, 1, 0:w], start=(j == 0), stop=(j == 3),
                                 skip_group_check=True)
                nc.tensor.matmul(s_ps[0:2, qs:T], onescol2a,
                                 pt[:, 0, 0:w], start=(j == 0), stop=False,
                                 skip_group_check=True)
                nc.tensor.matmul(s_ps[0:2, qs:T], onescol2b,
                                 pt[:, 1, 0:w], start=False, stop=(j == 3),
                                 skip_group_check=True)
            # softmax denominator: r = exp(-ln(sum)) broadcast
            nc.scalar.activation(lnr[0:2, :], s_ps[0:2, :], AF.Ln)
            rb = ps_s.tile([128, 1024], F32, tag="s")
            nc.tensor.matmul(rb[:, 0:T], bc2, lnr, start=True, stop=True)
            r_sb = sb_r.tile([128, T], F32, tag="r")
            nc.scalar.activation(r_sb, rb[:, 0:T], AF.Exp, scale=-1.0)
            ot = sb_ot.tile([128, T], BF16, tag="ot")
            nc.vector.tensor_mul(ot, o_ps, r_sb)
            oTs.append(ot)

        for tt in range(NTT):
            yp = ps_s.tile([128, 1024], F32, tag="s")
            for p in range(NPAIR):
                nc.tensor.matmul(yp[:, 0:C], oTs[p][:, ts(tt, 128)], wo[p],
                                 start=(p == 0), stop=(p == NPAIR - 1))
            yt = sb_y.tile([128, C], F32, tag="y")
            if tt % 2 == 0:
                nc.vector.tensor_copy(out=yt, in_=yp[:, 0:C])
            else:
                nc.scalar.activation(yt, yp[:, 0:C], AF.Copy)
            nc.sync.dma_start(out=y_d[b, ts(tt, 128), :], in_=yt)

    # software pipeline, 2 batches of lookahead: attention of batch b is
    # emitted BEFORE the projection phase of batch b+2, so the scheduler
    # prefers attention and uses the (already-emitted) front work of
    # batches b+1/b+2 as filler during exp/rope waits
    states = [emit_front(0), emit_front(1) if bl > 1 else None]
    for b in range(bl):
        emit_back(b, *states[b % 2])
        if b + 2 < bl:
            states[b % 2] = emit_front(b + 2)

    ctx.close()


def make_host_consts(Wqkv, Wout, cos, sin):
    Wq = Wqkv[0:C].astype(np.float32) / np.sqrt(np.float32(HD))
    Wk = Wqkv[C:2 * C].astype(np.float32)
    Wv = Wqkv[2 * C:3 * C].astype(np.float32)

    wqkT = np.concatenate([Wq, Wk], axis=0).T.copy()  # [C, 2C]

    wvT = Wv.T.copy()
    woutT = Wout.astype(np.float32).T.copy()

    cosA = np.asarray(cos, np.float32).reshape(T, RD // 2)  # [T, 8]
    sinA = np.asarray(sin, np.float32).reshape(T, RD // 2)
    cmask = np.ones((128, T), np.float32)
    smask = np.zeros((128, T), np.float32)
    for base in (0, 64):
        for i in range(RD):
            cmask[base + i] = cosA[:, i % (RD // 2)]
            # row i (i<8) holds r1_out = r1*cos - r2*sin; the shuffle swaps
            # in r2, so the sin factor is negative there
            sgn = -1.0 if i < RD // 2 else 1.0
            smask[base + i] = sgn * sinA[:, i % (RD // 2)]

    kk, qq = np.meshgrid(np.arange(128), np.arange(128), indexing="ij")
    maskw = np.where(qq >= kk, 0.0, NEG).astype(ml_dtypes.bfloat16)
    identb = np.eye(128, dtype=ml_dtypes.bfloat16)
    identf = np.eye(128, dtype=np.float32)
    onescol2a = np.zeros((128, 2), ml_dtypes.bfloat16)
    onescol2a[:, 0] = 1.0
    onescol2b = np.zeros((128, 2), ml_dtypes.bfloat16)
    onescol2b[:, 1] = 1.0
    bc2 = np.zeros((128, 128), np.float32)
    bc2[0, 0:64] = 1.0
    bc2[1, 64:128] = 1.0

    return dict(wqkT=wqkT, wvT=wvT, woutT=woutT,
                cmask=cmask, smask=smask, maskw=maskw, identb=identb,
                identf=identf, onescol2a=onescol2a, onescol2b=onescol2b,
                bc2=bc2)


_CACHE = {}


def prepare(x, Wqkv, Wout, cos, sin):
    if "nc" not in _CACHE:
        _CACHE["nc"] = build_program()
    nc = _CACHE["nc"]
    consts = make_host_consts(np.asarray(Wqkv), np.asarray(Wout), cos, sin)
    x = np.ascontiguousarray(np.asarray(x, np.float32))
    in_maps = []
    for c in range(NCORES):
        m = dict(consts)
        m["x"] = x[c * BL:(c + 1) * BL]
        in_maps.append(m)
    return nc, in_maps


def run(x, Wqkv, Wout, cos, sin, trace=False):
    nc, in_maps = prepare(x, Wqkv, Wout, cos, sin)
    res = run_bass_kernel_spmd(
        nc, in_maps, core_ids=list(range(NCORES)), trace=trace
    )
    y = np.concatenate([res.results[c]["y"] for c in range(NCORES)], axis=0)
    return y, res


def kernel(x, Wqkv, Wout, cos, sin):
    y, _ = run(x, Wqkv, Wout, cos, sin, trace=False)
    return y

